# revision 4
# baseline (speedup 1.0000x reference)
"""Bass/Trainium2 kernel for nn_BasicQuantumAttention (B=4, L=2048, d=512, 8 cores).

Sharding: core (b, s) = batch b, stream s (real/imag). Per core, fully fused
single-pass bf16 pipeline (f32 PSUM accumulation everywhere):
  - projection computes q^T, k^T (via W^T as lhsT over x^T) and v directly
    into resident SBUF -- no transposes, no DRAM round-trip
  - block-sparse masked attention (compile-time tile skipping over the union
    of the two masks; all 8 cores share one program)
  - partial out-projection y^T_part = W_out^T[stream rows].T @ O_norm^T
Host sums the two partial y^T per batch and untransposes.

Projection slices (256 tokens) are interleaved with attention/out-projection
blocks in emission order so softmax reciprocal chains hide under projection
matmuls and the PE never idles between phases.
"""
import sys

sys.path.insert(0, "/opt/trn_rl_repo")

import numpy as np
from ml_dtypes import bfloat16

import concourse.bass as bass
import concourse.tile as tile
from concourse import bacc, bass_isa, mybir
from concourse.bass_utils import run_bass_kernel_spmd

B, L, D = 4, 2048, 512
C6 = 6 * D            # 3072 input features
CT = C6 // 128        # 24 contraction tiles
NT = L // 128         # 16 token tiles of 128
NS = L // 256         # 8 projection slices of 256 tokens
QS = L // 512         # 4 query slices of 512 (out-projection granularity)
QB = L // 128         # 16 query blocks of 128 (attention granularity)
KT = L // 128         # 16 key tiles of 128
F32 = mybir.dt.float32
BF = mybir.dt.bfloat16
SCALE = float(D) ** -0.5
IDENT = mybir.ActivationFunctionType.Identity
EXP = mybir.ActivationFunctionType.Exp

# feature offsets inside qkv = [q_r q_i k_r k_i v_r v_i] (each D wide)
_Q_OFF = {0: 0 * D, 1: 1 * D}
_K_OFF = {0: 2 * D, 1: 3 * D}
_V_OFF = {0: 4 * D, 1: 5 * D}

LAST_RESULTS = None  # for test harness introspection


def build_program(kept, needs_mask, slot_of, width, n_slots, n_reps=1):
    """kept: {qs: [kt,...]} union keep lists; needs_mask: set[(qs,kt)];
    slot_of: {(qs,kt): slot index}; n_slots >= 1."""
    nc = bacc.Bacc(None, target_bir_lowering=False, debug=False)

    x_t = nc.dram_tensor("x_t", [128, NS, CT, 256], BF, kind="ExternalInput")
    wq = nc.dram_tensor("wq", [128, CT, D], BF, kind="ExternalInput")
    wk = nc.dram_tensor("wk", [128, CT, D], BF, kind="ExternalInput")
    wv = nc.dram_tensor("wv", [128, CT, D], BF, kind="ExternalInput")
    wo = nc.dram_tensor("wo", [128, 4, 2 * D], BF, kind="ExternalInput")
    b_qk = nc.dram_tensor("b_qk", [128, 8], F32, kind="ExternalInput")
    b_y = nc.dram_tensor("b_y", [128, 8], F32, kind="ExternalInput")
    mask_t = nc.dram_tensor("mask_t", [n_slots, 128, 128], BF, kind="ExternalInput")
    y = nc.dram_tensor("y", [2 * D, L], F32, kind="ExternalOutput")

    with tile.TileContext(nc) as tc, \
         nc.allow_low_precision(reason="bf16 matmul inputs, f32 accumulation"), \
         tc.tile_pool(name="consts", bufs=1) as consts, \
         tc.tile_pool(name="xin", bufs=3) as xp, \
         tc.tile_pool(name="pp", bufs=2, space="PSUM") as pp, \
         tc.tile_pool(name="sy", bufs=2, space="PSUM") as syp, \
         tc.tile_pool(name="op", bufs=1, space="PSUM") as opp, \
         tc.tile_pool(name="pt", bufs=4) as ptp, \
         tc.tile_pool(name="mk", bufs=8) as mkp, \
         tc.tile_pool(name="ot", bufs=2) as otp, \
         tc.tile_pool(name="dc", bufs=2) as dcp, \
         tc.tile_pool(name="oc", bufs=2) as ocp, \
         tc.tile_pool(name="rc", bufs=2) as rcp, \
         tc.tile_pool(name="rb", bufs=2) as rbp, \
         tc.tile_pool(name="yo", bufs=3) as yop:
        wq_sb = consts.tile([128, CT, D], BF)
        wk_sb = consts.tile([128, CT, D], BF)
        wv_sb = consts.tile([128, CT, D], BF)
        wo_sb = consts.tile([128, 4, 2 * D], BF)
        bqk_sb = consts.tile([128, 8], F32)
        by_sb = consts.tile([128, 8], F32)

        def load_x(ns2, eng=None, splits=(12, 24)):
            xt = xp.tile([128, CT, 256], BF, tag="x", name="xt")
            h0 = 0
            for h1 in splits:
                (eng or nc.sync).dma_start(
                    out=xt[:, h0:h1, :],
                    in_=x_t[:, ns2, h0:h1, :])
                h0 = h1
            return xt

        # x0/x1 go out on the Act queue (idle at startup) so SP and Pool can
        # stream weight chunks from t=0; all run ahead of first use. x0's
        # first piece is small so the very first matmul unblocks early.
        x_tiles = {0: load_x(0, eng=nc.scalar, splits=(4, 12, 24)),
                   1: load_x(1, eng=nc.scalar)}

        # bias first (it gates the first PSUM evacuation), then weights in
        # chunks alternating Pool/SP so arrival order tracks the ct
        # consumption order; non-critical consts follow the weights
        nc.gpsimd.dma_start(out=bqk_sb, in_=b_qk[:, :])
        wq_chunks = [(0, 2), (2, 4), (4, 8), (8, 12), (12, 16), (16, 20),
                     (20, 24)]
        for i, (c0, c1) in enumerate(wq_chunks):
            eng = nc.gpsimd if i % 2 == 0 else nc.sync
            eng.dma_start(out=wq_sb[:, c0:c1, :], in_=wq[:, c0:c1, :])
        for w_dram, w_sb in ((wk, wk_sb), (wv, wv_sb)):
            for i, c0 in enumerate(range(0, CT, 4)):
                eng = nc.gpsimd if i % 2 == 0 else nc.sync
                eng.dma_start(out=w_sb[:, c0:c0 + 4, :],
                              in_=w_dram[:, c0:c0 + 4, :])
        nc.gpsimd.dma_start(out=wo_sb, in_=wo[:, :, :])
        nc.gpsimd.dma_start(out=by_sb, in_=b_y[:, :])
        # warmup: trigger the one-time activation function-table load off
        # the critical path, before the first PSUM evacuation needs it
        warm = consts.tile([1, 1], F32)
        nc.scalar.activation(out=warm, in_=bqk_sb[0:1, 0:1], func=EXP)

        qT_sb = consts.tile([128, 4, L], BF, name="qT")
        kT_sb = consts.tile([128, 4, L], BF, name="kT")
        v_sb = consts.tile([128, NT, D], BF, name="v")

        for _rep in range(n_reps):
            if _rep > 0:
                x_tiles = {0: load_x(0), 1: load_x(1)}

            def emit_P(ns2):
                xt = x_tiles.pop(ns2)
                if ns2 + 2 < NS:
                    x_tiles[ns2 + 2] = load_x(ns2 + 2)
                for which, w_sb, dst in (("q", wq_sb, qT_sb),
                                         ("k", wk_sb, kT_sb)):
                    for dt in range(4):
                        ps = pp.tile([128, D], F32, tag="pp")
                        for ct in range(CT):
                            nc.tensor.matmul(
                                ps[:, 0:256],
                                w_sb[:, ct, dt * 128:(dt + 1) * 128],
                                xt[:, ct, :],
                                start=(ct == 0), stop=(ct == CT - 1))
                        bc = dt if which == "q" else 4 + dt
                        nc.scalar.activation(
                            out=dst[:, dt, ns2 * 256:(ns2 + 1) * 256],
                            in_=ps[:, 0:256], func=IDENT,
                            bias=bqk_sb[:, bc:bc + 1])
                for nt2 in range(2):
                    nt = ns2 * 2 + nt2
                    ps = pp.tile([128, D], F32, tag="pp")
                    for ct in range(CT):
                        nc.tensor.matmul(
                            ps[:, :], xt[:, ct, nt2 * 128:(nt2 + 1) * 128],
                            wv_sb[:, ct, :],
                            start=(ct == 0), stop=(ct == CT - 1))
                    nc.scalar.copy(out=v_sb[:, nt, :], in_=ps)

            otqs = {}

            def emit_A(qs, filler=None):
                # 4 query blocks of 128, each with its own softmax chain,
                # normalized into one [128, 4dvt, 512] tile for emit_O.
                # filler(j) emits independent PE work between blocks so the
                # per-block softmax chains stay hidden even with no
                # projection work left (used for the final A group).
                otq = otp.tile([128, 4, 512], BF, tag="otq", name="otq")
                otqs[qs] = otq
                for qb in range(4 * qs, 4 * qs + 4):
                    if filler is not None and qb > 4 * qs:
                        filler(qb - 4 * qs - 1)
                    klist = kept[qb]
                    # PSUM is bank-granular (2 KB): pack the four [128,128]
                    # AV accumulators and the rotating score tiles into
                    # [128,512] bank tiles, addressed by 128-col slices
                    # one open accumulation group per PSUM bank: the four AV
                    # accumulators and each rotating score tile get their own
                    # bank-aligned tiles
                    ops = [opp.tile([128, 128], F32, tag=f"o{dvt}",
                                    name=f"ops{dvt}") for dvt in range(4)]
                    dacc = dcp.tile([128, 128], F32, name="dacc")
                    # software-pipelined: scores for tile i+1 are emitted
                    # before the AV matmuls of tile i, so the PE streams
                    # through the exp->mask latency of each tile
                    n_k = len(klist)
                    pend = []  # (i, kt, pT, c0, cw) awaiting AV matmuls

                    def emit_AV(i, kt, pT, c0, cw):
                        for dvt in range(4):
                            nc.tensor.matmul(
                                ops[dvt][:, c0:c0 + cw],
                                v_sb[:, kt, dvt * 128:(dvt + 1) * 128],
                                pT[:, c0:c0 + cw],
                                start=(i == 0), stop=(i == n_k - 1))

                    for i, kt in enumerate(klist):
                        c0, cw = width[(qb, kt)]
                        sps = syp.tile([128, 128], F32, tag="sy", name="sps")
                        for dt in range(4):
                            nc.tensor.matmul(
                                sps[:, c0:c0 + cw],
                                kT_sb[:, dt, kt * 128:(kt + 1) * 128],
                                qT_sb[:, dt, qb * 128 + c0:qb * 128 + c0 + cw],
                                start=(dt == 0), stop=(dt == 3))
                        pT = ptp.tile([128, 128], BF, name="pT")
                        nc.scalar.activation(out=pT[:, c0:c0 + cw],
                                             in_=sps[:, c0:c0 + cw], func=EXP,
                                             scale=SCALE)
                        if (qb, kt) in needs_mask:
                            mt = mkp.tile([128, 128], BF, name="mt")
                            nc.sync.dma_start(
                                out=mt[:, c0:c0 + cw],
                                in_=mask_t[slot_of[(qb, kt)], :, c0:c0 + cw])
                            nc.vector.tensor_mul(pT[:, c0:c0 + cw],
                                                 pT[:, c0:c0 + cw],
                                                 mt[:, c0:c0 + cw])
                        if i == 0:
                            # diagonal tile: always full width, initializes
                            # the whole accumulator
                            nc.vector.tensor_copy(out=dacc, in_=pT)
                        else:
                            nc.vector.tensor_add(dacc[:, c0:c0 + cw],
                                                 dacc[:, c0:c0 + cw],
                                                 pT[:, c0:c0 + cw])
                        pend.append((i, kt, pT, c0, cw))
                        if len(pend) > 2:
                            emit_AV(*pend.pop(0))
                    for p_ in pend:
                        emit_AV(*p_)
                    # evacuate the AV accumulators with plain copies first:
                    # this releases the PSUM banks for the next block without
                    # waiting on the reciprocal chain
                    oc = [ocp.tile([128, 128], BF, tag=f"c{dvt}",
                                   name=f"oc{dvt}") for dvt in range(4)]
                    for dvt in range(4):
                        nc.vector.tensor_copy(out=oc[dvt], in_=ops[dvt][:, :])
                    # denominator: all-reduce across partitions on gpsimd
                    # (reduce + broadcast in one op, PE stays out of it),
                    # then reciprocal on DVE and deferred normalization
                    den = rcp.tile([128, 128], F32, name="den")
                    nc.gpsimd.partition_all_reduce(
                        den, dacc, channels=128,
                        reduce_op=bass_isa.ReduceOp.add)
                    rb = rbp.tile([128, 128], F32, name="rb")
                    nc.vector.reciprocal(rb, den)
                    qo = (qb - 4 * qs) * 128
                    for dvt in range(4):
                        nc.vector.tensor_mul(
                            otq[:, dvt, qo:qo + 128], oc[dvt], rb)

            def emit_O(qs, gts=range(8)):
                pieces = 1
                for gt in gts:
                    yps = pp.tile([128, D], F32, tag="pp", name="yps")
                    for dvt in range(4):
                        nc.tensor.matmul(
                            yps[:, :], wo_sb[:, dvt, gt * 128:(gt + 1) * 128],
                            otqs[qs][:, dvt, :],
                            start=(dvt == 0), stop=(dvt == 3))
                    y_sb = yop.tile([128, 512], F32, name="ysb")
                    w = 512 // pieces
                    for pc in range(pieces):
                        nc.scalar.activation(
                            out=y_sb[:, pc * w:(pc + 1) * w],
                            in_=yps[:, pc * w:(pc + 1) * w], func=IDENT,
                            bias=by_sb[:, gt:gt + 1])
                        nc.sync.dma_start(
                            out=y[gt * 128:(gt + 1) * 128,
                                  qs * 512 + pc * w:qs * 512 + (pc + 1) * w],
                            in_=y_sb[:, pc * w:(pc + 1) * w])

            emit_P(0); emit_P(1)
            emit_A(0)
            emit_P(2); emit_P(3)
            emit_A(1); emit_O(0)
            emit_P(4); emit_P(5)
            emit_A(2); emit_O(1)
            emit_P(6); emit_P(7)
            emit_A(3); emit_O(2); emit_O(3)

    nc.compile()
    return nc


def _prep_masks(mask_real, mask_imag):
    """Compile-time tile analysis at [128 keys, 128 queries] granularity
    over the union of the two stream masks + per-core mask slot data.

    Each kept (qb, kt) tile carries a query-column range (c0, cw): full
    width (0, 128) or one 64-wide half when the union mask is empty on the
    other half. The always-full-width diagonal tile is moved to the front
    of each klist so the first AV matmul of a block opens the PSUM
    accumulation group over the full bank."""
    mts = [np.ascontiguousarray(np.asarray(m).T) for m in (mask_real, mask_imag)]
    kept = {}
    needs_mask = set()
    slot_of = {}
    width = {}
    slots = []  # (qb, kt)
    for qb in range(QB):
        klist = []
        for kt in range(KT):
            subs = [m[kt * 128:(kt + 1) * 128, qb * 128:(qb + 1) * 128] for m in mts]
            lo = any(s[:, :64].any() for s in subs)
            hi = any(s[:, 64:].any() for s in subs)
            if not (lo or hi):
                continue
            klist.append(kt)
            if kt == qb or (lo and hi):
                c0, cw = 0, 128
            elif lo:
                c0, cw = 0, 64
            else:
                c0, cw = 64, 64
            width[(qb, kt)] = (c0, cw)
            if not all(s[:, c0:c0 + cw].all() for s in subs):
                needs_mask.add((qb, kt))
                slot_of[(qb, kt)] = len(slots)
                slots.append((qb, kt))
        # diagonal tile first: it is always kept and always full width
        klist.remove(qb)
        klist.insert(0, qb)
        kept[qb] = klist
    n_slots = max(1, len(slots))
    mask_data = []
    for s in range(2):
        md = np.ones((n_slots, 128, 128), bfloat16)
        for i, (qb, kt) in enumerate(slots):
            md[i] = mts[s][kt * 128:(kt + 1) * 128,
                           qb * 128:(qb + 1) * 128].astype(bfloat16)
        mask_data.append(md)
    return kept, needs_mask, slot_of, width, n_slots, mask_data


def kernel(q_real, q_imag, k_real, k_imag, v_real, v_imag,
           W_qkv, b_qkv, W_out, b_out, mask_real, mask_imag, _trace=False):
    global LAST_RESULTS
    args = [np.asarray(a) for a in (q_real, q_imag, k_real, k_imag, v_real, v_imag)]
    W_qkv = np.asarray(W_qkv, np.float32)
    b_qkv = np.asarray(b_qkv, np.float32)
    W_out = np.asarray(W_out, np.float32)
    b_out = np.asarray(b_out, np.float32)

    kept, needs_mask, slot_of, width, n_slots, mask_data = _prep_masks(
        mask_real, mask_imag)
    nc = build_program(kept, needs_mask, slot_of, width, n_slots)

    # x^T per batch, partition-major: [128, CT, L]
    x_ts = []
    for b in range(B):
        xb = np.concatenate([a[b] for a in args], axis=1)           # [L, 6D]
        xt = xb.T.astype(bfloat16)                                  # [6D, L]
        x_ts.append(np.ascontiguousarray(
            xt.reshape(CT, 128, NS, 256).transpose(1, 2, 0, 3)))    # [128,NS,CT,256]

    W6T = W_qkv.T  # [c, f]
    W2T = W_out.T  # [f=2D, g=2D]
    wqs, wks, wvs, wos, b_qks, b_ys = [], [], [], [], [], []
    for s in range(2):
        wqs.append(np.ascontiguousarray(
            W6T[:, _Q_OFF[s]:_Q_OFF[s] + D].astype(bfloat16)
            .reshape(CT, 128, D).transpose(1, 0, 2)))               # [128,CT,D]
        wks.append(np.ascontiguousarray(
            W6T[:, _K_OFF[s]:_K_OFF[s] + D].astype(bfloat16)
            .reshape(CT, 128, D).transpose(1, 0, 2)))
        wvs.append(np.ascontiguousarray(
            W6T[:, _V_OFF[s]:_V_OFF[s] + D].astype(bfloat16)
            .reshape(CT, 128, D).transpose(1, 0, 2)))
        wos.append(np.ascontiguousarray(
            W2T[s * D:(s + 1) * D, :].astype(bfloat16)
            .reshape(4, 128, 2 * D).transpose(1, 0, 2)))            # [128,4,1024]
        bq = b_qkv[_Q_OFF[s]:_Q_OFF[s] + D].reshape(4, 128).T
        bk = b_qkv[_K_OFF[s]:_K_OFF[s] + D].reshape(4, 128).T
        b_qks.append(np.ascontiguousarray(
            np.concatenate([bq, bk], axis=1), dtype=np.float32))    # [128, 8]
        if s == 0:
            b_v_cat = np.concatenate([b_qkv[_V_OFF[0]:_V_OFF[0] + D],
                                      b_qkv[_V_OFF[1]:_V_OFF[1] + D]])
            b_eff = (W_out @ b_v_cat + b_out).astype(np.float32)
            b_ys.append(np.ascontiguousarray(b_eff.reshape(8, 128).T))
        else:
            b_ys.append(np.zeros((128, 8), np.float32))

    in_maps = []
    for core in range(8):
        b, s = core // 2, core % 2
        in_maps.append({
            "x_t": x_ts[b], "wq": wqs[s], "wk": wks[s], "wv": wvs[s],
            "wo": wos[s], "b_qk": b_qks[s], "b_y": b_ys[s],
            "mask_t": mask_data[s],
        })

    res = run_bass_kernel_spmd(nc, in_maps, core_ids=list(range(8)), trace=_trace)
    LAST_RESULTS = res

    out_real = np.empty((B, L, D), np.float32)
    out_imag = np.empty((B, L, D), np.float32)
    for b in range(B):
        yt = res.results[2 * b]["y"] + res.results[2 * b + 1]["y"]  # [2D, L]
        yb = yt.T                                                   # [L, 2D]
        out_real[b] = yb[:, :D]
        out_imag[b] = yb[:, D:]
    return out_real, out_imag


# revision 5
# speedup vs baseline: 1.0013x; 1.0013x over previous
"""Bass/Trainium2 kernel for nn_BasicQuantumAttention (B=4, L=2048, d=512, 8 cores).

Sharding: core (b, s) = batch b, stream s (real/imag). Per core, fully fused
single-pass bf16 pipeline (f32 PSUM accumulation everywhere):
  - projection computes q^T, k^T (via W^T as lhsT over x^T) and v directly
    into resident SBUF -- no transposes, no DRAM round-trip
  - block-sparse masked attention (compile-time tile skipping over the union
    of the two masks; all 8 cores share one program)
  - partial out-projection y^T_part = W_out^T[stream rows].T @ O_norm^T
Host sums the two partial y^T per batch and untransposes.

Projection slices (256 tokens) are interleaved with attention/out-projection
blocks in emission order so softmax reciprocal chains hide under projection
matmuls and the PE never idles between phases.
"""
import sys

sys.path.insert(0, "/opt/trn_rl_repo")

import numpy as np
from ml_dtypes import bfloat16

import concourse.bass as bass
import concourse.tile as tile
from concourse import bacc, bass_isa, mybir
from concourse.bass_utils import run_bass_kernel_spmd

B, L, D = 4, 2048, 512
C6 = 6 * D            # 3072 input features
CT = C6 // 128        # 24 contraction tiles
NT = L // 128         # 16 token tiles of 128
NS = L // 256         # 8 projection slices of 256 tokens
QS = L // 512         # 4 query slices of 512 (out-projection granularity)
QB = L // 128         # 16 query blocks of 128 (attention granularity)
KT = L // 128         # 16 key tiles of 128
F32 = mybir.dt.float32
BF = mybir.dt.bfloat16
SCALE = float(D) ** -0.5
IDENT = mybir.ActivationFunctionType.Identity
EXP = mybir.ActivationFunctionType.Exp

# feature offsets inside qkv = [q_r q_i k_r k_i v_r v_i] (each D wide)
_Q_OFF = {0: 0 * D, 1: 1 * D}
_K_OFF = {0: 2 * D, 1: 3 * D}
_V_OFF = {0: 4 * D, 1: 5 * D}

LAST_RESULTS = None  # for test harness introspection


def build_program(kept, needs_mask, slot_of, width, n_slots, n_reps=1):
    """kept: {qs: [kt,...]} union keep lists; needs_mask: set[(qs,kt)];
    slot_of: {(qs,kt): slot index}; n_slots >= 1."""
    nc = bacc.Bacc(None, target_bir_lowering=False, debug=False)

    x_t = nc.dram_tensor("x_t", [128, NS, CT, 256], BF, kind="ExternalInput")
    wq = nc.dram_tensor("wq", [128, CT, D], BF, kind="ExternalInput")
    wk = nc.dram_tensor("wk", [128, CT, D], BF, kind="ExternalInput")
    wv = nc.dram_tensor("wv", [128, CT, D], BF, kind="ExternalInput")
    wo = nc.dram_tensor("wo", [128, 4, 2 * D], BF, kind="ExternalInput")
    b_qk = nc.dram_tensor("b_qk", [128, 8], F32, kind="ExternalInput")
    b_y = nc.dram_tensor("b_y", [128, 8], F32, kind="ExternalInput")
    mask_t = nc.dram_tensor("mask_t", [n_slots, 128, 128], BF, kind="ExternalInput")
    y = nc.dram_tensor("y", [2 * D, L], F32, kind="ExternalOutput")

    with tile.TileContext(nc) as tc, \
         nc.allow_low_precision(reason="bf16 matmul inputs, f32 accumulation"), \
         tc.tile_pool(name="consts", bufs=1) as consts, \
         tc.tile_pool(name="xin", bufs=3) as xp, \
         tc.tile_pool(name="pp", bufs=2, space="PSUM") as pp, \
         tc.tile_pool(name="sy", bufs=2, space="PSUM") as syp, \
         tc.tile_pool(name="op", bufs=1, space="PSUM") as opp, \
         tc.tile_pool(name="pt", bufs=4) as ptp, \
         tc.tile_pool(name="mk", bufs=8) as mkp, \
         tc.tile_pool(name="ot", bufs=2) as otp, \
         tc.tile_pool(name="dc", bufs=2) as dcp, \
         tc.tile_pool(name="oc", bufs=2) as ocp, \
         tc.tile_pool(name="rc", bufs=2) as rcp, \
         tc.tile_pool(name="rb", bufs=2) as rbp, \
         tc.tile_pool(name="yo", bufs=3) as yop:
        wq_sb = consts.tile([128, CT, D], BF)
        wk_sb = consts.tile([128, CT, D], BF)
        wv_sb = consts.tile([128, CT, D], BF)
        wo_sb = consts.tile([128, 4, 2 * D], BF)
        bqk_sb = consts.tile([128, 8], F32)
        by_sb = consts.tile([128, 8], F32)

        def load_x(ns2, eng=None, splits=(12, 24)):
            xt = xp.tile([128, CT, 256], BF, tag="x", name="xt")
            h0 = 0
            for h1 in splits:
                (eng or nc.sync).dma_start(
                    out=xt[:, h0:h1, :],
                    in_=x_t[:, ns2, h0:h1, :])
                h0 = h1
            return xt

        # x0/x1 go out on the Act queue (idle at startup) so SP and Pool can
        # stream weight chunks from t=0; all run ahead of first use. x0's
        # first piece is small so the very first matmul unblocks early.
        x_tiles = {0: load_x(0, eng=nc.scalar, splits=(4, 12, 24)),
                   1: load_x(1, eng=nc.scalar)}

        # bias first (it gates the first PSUM evacuation), then weights in
        # chunks alternating Pool/SP so arrival order tracks the ct
        # consumption order; non-critical consts follow the weights
        nc.gpsimd.dma_start(out=bqk_sb, in_=b_qk[:, :])
        wq_chunks = [(0, 2), (2, 4), (4, 8), (8, 12), (12, 16), (16, 20),
                     (20, 24)]
        for i, (c0, c1) in enumerate(wq_chunks):
            eng = nc.gpsimd if i % 2 == 0 else nc.sync
            eng.dma_start(out=wq_sb[:, c0:c1, :], in_=wq[:, c0:c1, :])
        for w_dram, w_sb in ((wk, wk_sb), (wv, wv_sb)):
            for i, c0 in enumerate(range(0, CT, 4)):
                eng = nc.gpsimd if i % 2 == 0 else nc.sync
                eng.dma_start(out=w_sb[:, c0:c0 + 4, :],
                              in_=w_dram[:, c0:c0 + 4, :])
        nc.gpsimd.dma_start(out=wo_sb, in_=wo[:, :, :])
        nc.gpsimd.dma_start(out=by_sb, in_=b_y[:, :])
        # warmup: trigger the one-time activation function-table load off
        # the critical path, before the first PSUM evacuation needs it
        warm = consts.tile([1, 1], F32)
        nc.scalar.activation(out=warm, in_=bqk_sb[0:1, 0:1], func=EXP)

        qT_sb = consts.tile([128, 4, L], BF, name="qT")
        kT_sb = consts.tile([128, 4, L], BF, name="kT")
        v_sb = consts.tile([128, NT, D], BF, name="v")

        for _rep in range(n_reps):
            if _rep > 0:
                x_tiles = {0: load_x(0), 1: load_x(1)}

            def emit_P(ns2):
                xt = x_tiles.pop(ns2)
                if ns2 + 2 < NS:
                    x_tiles[ns2 + 2] = load_x(ns2 + 2)
                for which, w_sb, dst in (("q", wq_sb, qT_sb),
                                         ("k", wk_sb, kT_sb)):
                    for dt in range(4):
                        ps = pp.tile([128, D], F32, tag="pp")
                        for ct in range(CT):
                            nc.tensor.matmul(
                                ps[:, 0:256],
                                w_sb[:, ct, dt * 128:(dt + 1) * 128],
                                xt[:, ct, :],
                                start=(ct == 0), stop=(ct == CT - 1))
                        bc = dt if which == "q" else 4 + dt
                        nc.scalar.activation(
                            out=dst[:, dt, ns2 * 256:(ns2 + 1) * 256],
                            in_=ps[:, 0:256], func=IDENT,
                            bias=bqk_sb[:, bc:bc + 1])
                for nt2 in range(2):
                    nt = ns2 * 2 + nt2
                    ps = pp.tile([128, D], F32, tag="pp")
                    for ct in range(CT):
                        nc.tensor.matmul(
                            ps[:, :], xt[:, ct, nt2 * 128:(nt2 + 1) * 128],
                            wv_sb[:, ct, :],
                            start=(ct == 0), stop=(ct == CT - 1))
                    nc.scalar.copy(out=v_sb[:, nt, :], in_=ps)

            otqs = {}

            def emit_A(qs, filler=None):
                # 4 query blocks of 128, each with its own softmax chain,
                # normalized into one [128, 4dvt, 512] tile for emit_O.
                # filler(j) emits independent PE work between blocks so the
                # per-block softmax chains stay hidden even with no
                # projection work left (used for the final A group).
                otq = otp.tile([128, 4, 512], BF, tag="otq", name="otq")
                otqs[qs] = otq
                for qb in range(4 * qs, 4 * qs + 4):
                    if filler is not None and qb > 4 * qs:
                        filler(qb - 4 * qs - 1)
                    klist = kept[qb]
                    # PSUM is bank-granular (2 KB): pack the four [128,128]
                    # AV accumulators and the rotating score tiles into
                    # [128,512] bank tiles, addressed by 128-col slices
                    # one open accumulation group per PSUM bank: the four AV
                    # accumulators and each rotating score tile get their own
                    # bank-aligned tiles
                    ops = [opp.tile([128, 128], F32, tag=f"o{dvt}",
                                    name=f"ops{dvt}") for dvt in range(4)]
                    dacc = dcp.tile([128, 128], F32, name="dacc")
                    # software-pipelined: scores for tile i+1 are emitted
                    # before the AV matmuls of tile i, so the PE streams
                    # through the exp->mask latency of each tile
                    n_k = len(klist)
                    pend = []  # (i, kt, pT, c0, cw) awaiting AV matmuls

                    def emit_AV(i, kt, pT, c0, cw):
                        for dvt in range(4):
                            nc.tensor.matmul(
                                ops[dvt][:, c0:c0 + cw],
                                v_sb[:, kt, dvt * 128:(dvt + 1) * 128],
                                pT[:, c0:c0 + cw],
                                start=(i == 0), stop=(i == n_k - 1))

                    for i, kt in enumerate(klist):
                        c0, cw = width[(qb, kt)]
                        sps = syp.tile([128, 128], F32, tag="sy", name="sps")
                        for dt in range(4):
                            nc.tensor.matmul(
                                sps[:, c0:c0 + cw],
                                kT_sb[:, dt, kt * 128:(kt + 1) * 128],
                                qT_sb[:, dt, qb * 128 + c0:qb * 128 + c0 + cw],
                                start=(dt == 0), stop=(dt == 3))
                        pT = ptp.tile([128, 128], BF, name="pT")
                        nc.scalar.activation(out=pT[:, c0:c0 + cw],
                                             in_=sps[:, c0:c0 + cw], func=EXP,
                                             scale=SCALE)
                        if (qb, kt) in needs_mask:
                            mt = mkp.tile([128, 128], BF, name="mt")
                            nc.sync.dma_start(
                                out=mt[:, c0:c0 + cw],
                                in_=mask_t[slot_of[(qb, kt)], :, c0:c0 + cw])
                            nc.vector.tensor_mul(pT[:, c0:c0 + cw],
                                                 pT[:, c0:c0 + cw],
                                                 mt[:, c0:c0 + cw])
                        if i == 0:
                            # diagonal tile: always full width, initializes
                            # the whole accumulator
                            nc.vector.tensor_copy(out=dacc, in_=pT)
                        else:
                            nc.vector.tensor_add(dacc[:, c0:c0 + cw],
                                                 dacc[:, c0:c0 + cw],
                                                 pT[:, c0:c0 + cw])
                        pend.append((i, kt, pT, c0, cw))
                        if len(pend) > 2:
                            emit_AV(*pend.pop(0))
                    for p_ in pend:
                        emit_AV(*p_)
                    # evacuate the AV accumulators with plain copies first:
                    # this releases the PSUM banks for the next block without
                    # waiting on the reciprocal chain
                    oc = [ocp.tile([128, 128], BF, tag=f"c{dvt}",
                                   name=f"oc{dvt}") for dvt in range(4)]
                    for dvt in range(4):
                        nc.vector.tensor_copy(out=oc[dvt], in_=ops[dvt][:, :])
                    # denominator: all-reduce across partitions on gpsimd
                    # (reduce + broadcast in one op, PE stays out of it),
                    # then reciprocal on DVE and deferred normalization
                    den = rcp.tile([128, 128], F32, name="den")
                    nc.gpsimd.partition_all_reduce(
                        den, dacc, channels=128,
                        reduce_op=bass_isa.ReduceOp.add)
                    rb = rbp.tile([128, 128], F32, name="rb")
                    nc.vector.reciprocal(rb, den)
                    qo = (qb - 4 * qs) * 128
                    for dvt in range(4):
                        nc.vector.tensor_mul(
                            otq[:, dvt, qo:qo + 128], oc[dvt], rb)

            def emit_O(qs, split=False):
                # split=True accumulates each yps tile in four sequential
                # 128-col piece-groups (one open group per bank at a time),
                # so the first pieces only depend on the first attention
                # blocks' normalization — the last O block starts while the
                # final softmax chains are still draining
                for gt in range(8):
                    yps = pp.tile([128, D], F32, tag="pp", name="yps")
                    if split:
                        for qp in range(4):
                            for dvt in range(4):
                                nc.tensor.matmul(
                                    yps[:, qp * 128:(qp + 1) * 128],
                                    wo_sb[:, dvt, gt * 128:(gt + 1) * 128],
                                    otqs[qs][:, dvt, qp * 128:(qp + 1) * 128],
                                    start=(dvt == 0), stop=(dvt == 3))
                    else:
                        for dvt in range(4):
                            nc.tensor.matmul(
                                yps[:, :], wo_sb[:, dvt, gt * 128:(gt + 1) * 128],
                                otqs[qs][:, dvt, :],
                                start=(dvt == 0), stop=(dvt == 3))
                    y_sb = yop.tile([128, 512], F32, name="ysb")
                    nc.scalar.activation(out=y_sb, in_=yps, func=IDENT,
                                         bias=by_sb[:, gt:gt + 1])
                    nc.sync.dma_start(
                        out=y[gt * 128:(gt + 1) * 128,
                              qs * 512:(qs + 1) * 512],
                        in_=y_sb)

            emit_P(0); emit_P(1)
            emit_A(0)
            emit_P(2); emit_P(3)
            emit_A(1); emit_O(0)
            emit_P(4); emit_P(5)
            emit_A(2); emit_O(1)
            emit_P(6); emit_P(7)
            emit_A(3); emit_O(2, split=True); emit_O(3, split=True)

    nc.compile()
    return nc


def _prep_masks(mask_real, mask_imag):
    """Compile-time tile analysis at [128 keys, 128 queries] granularity
    over the union of the two stream masks + per-core mask slot data.

    Each kept (qb, kt) tile carries a query-column range (c0, cw): full
    width (0, 128) or one 64-wide half when the union mask is empty on the
    other half. The always-full-width diagonal tile is moved to the front
    of each klist so the first AV matmul of a block opens the PSUM
    accumulation group over the full bank."""
    mts = [np.ascontiguousarray(np.asarray(m).T) for m in (mask_real, mask_imag)]
    kept = {}
    needs_mask = set()
    slot_of = {}
    width = {}
    slots = []  # (qb, kt)
    for qb in range(QB):
        klist = []
        for kt in range(KT):
            subs = [m[kt * 128:(kt + 1) * 128, qb * 128:(qb + 1) * 128] for m in mts]
            lo = any(s[:, :64].any() for s in subs)
            hi = any(s[:, 64:].any() for s in subs)
            if not (lo or hi):
                continue
            klist.append(kt)
            if kt == qb or (lo and hi):
                c0, cw = 0, 128
            elif lo:
                c0, cw = 0, 64
            else:
                c0, cw = 64, 64
            width[(qb, kt)] = (c0, cw)
            if not all(s[:, c0:c0 + cw].all() for s in subs):
                needs_mask.add((qb, kt))
                slot_of[(qb, kt)] = len(slots)
                slots.append((qb, kt))
        # diagonal tile first: it is always kept and always full width
        klist.remove(qb)
        klist.insert(0, qb)
        kept[qb] = klist
    n_slots = max(1, len(slots))
    mask_data = []
    for s in range(2):
        md = np.ones((n_slots, 128, 128), bfloat16)
        for i, (qb, kt) in enumerate(slots):
            md[i] = mts[s][kt * 128:(kt + 1) * 128,
                           qb * 128:(qb + 1) * 128].astype(bfloat16)
        mask_data.append(md)
    return kept, needs_mask, slot_of, width, n_slots, mask_data


def kernel(q_real, q_imag, k_real, k_imag, v_real, v_imag,
           W_qkv, b_qkv, W_out, b_out, mask_real, mask_imag, _trace=False):
    global LAST_RESULTS
    args = [np.asarray(a) for a in (q_real, q_imag, k_real, k_imag, v_real, v_imag)]
    W_qkv = np.asarray(W_qkv, np.float32)
    b_qkv = np.asarray(b_qkv, np.float32)
    W_out = np.asarray(W_out, np.float32)
    b_out = np.asarray(b_out, np.float32)

    kept, needs_mask, slot_of, width, n_slots, mask_data = _prep_masks(
        mask_real, mask_imag)
    nc = build_program(kept, needs_mask, slot_of, width, n_slots)

    # x^T per batch, partition-major: [128, CT, L]
    x_ts = []
    for b in range(B):
        xb = np.concatenate([a[b] for a in args], axis=1)           # [L, 6D]
        xt = xb.T.astype(bfloat16)                                  # [6D, L]
        x_ts.append(np.ascontiguousarray(
            xt.reshape(CT, 128, NS, 256).transpose(1, 2, 0, 3)))    # [128,NS,CT,256]

    W6T = W_qkv.T  # [c, f]
    W2T = W_out.T  # [f=2D, g=2D]
    wqs, wks, wvs, wos, b_qks, b_ys = [], [], [], [], [], []
    for s in range(2):
        wqs.append(np.ascontiguousarray(
            W6T[:, _Q_OFF[s]:_Q_OFF[s] + D].astype(bfloat16)
            .reshape(CT, 128, D).transpose(1, 0, 2)))               # [128,CT,D]
        wks.append(np.ascontiguousarray(
            W6T[:, _K_OFF[s]:_K_OFF[s] + D].astype(bfloat16)
            .reshape(CT, 128, D).transpose(1, 0, 2)))
        wvs.append(np.ascontiguousarray(
            W6T[:, _V_OFF[s]:_V_OFF[s] + D].astype(bfloat16)
            .reshape(CT, 128, D).transpose(1, 0, 2)))
        wos.append(np.ascontiguousarray(
            W2T[s * D:(s + 1) * D, :].astype(bfloat16)
            .reshape(4, 128, 2 * D).transpose(1, 0, 2)))            # [128,4,1024]
        bq = b_qkv[_Q_OFF[s]:_Q_OFF[s] + D].reshape(4, 128).T
        bk = b_qkv[_K_OFF[s]:_K_OFF[s] + D].reshape(4, 128).T
        b_qks.append(np.ascontiguousarray(
            np.concatenate([bq, bk], axis=1), dtype=np.float32))    # [128, 8]
        if s == 0:
            b_v_cat = np.concatenate([b_qkv[_V_OFF[0]:_V_OFF[0] + D],
                                      b_qkv[_V_OFF[1]:_V_OFF[1] + D]])
            b_eff = (W_out @ b_v_cat + b_out).astype(np.float32)
            b_ys.append(np.ascontiguousarray(b_eff.reshape(8, 128).T))
        else:
            b_ys.append(np.zeros((128, 8), np.float32))

    in_maps = []
    for core in range(8):
        b, s = core // 2, core % 2
        in_maps.append({
            "x_t": x_ts[b], "wq": wqs[s], "wk": wks[s], "wv": wvs[s],
            "wo": wos[s], "b_qk": b_qks[s], "b_y": b_ys[s],
            "mask_t": mask_data[s],
        })

    res = run_bass_kernel_spmd(nc, in_maps, core_ids=list(range(8)), trace=_trace)
    LAST_RESULTS = res

    out_real = np.empty((B, L, D), np.float32)
    out_imag = np.empty((B, L, D), np.float32)
    for b in range(B):
        yt = res.results[2 * b]["y"] + res.results[2 * b + 1]["y"]  # [2D, L]
        yb = yt.T                                                   # [L, 2D]
        out_real[b] = yb[:, :D]
        out_imag[b] = yb[:, D:]
    return out_real, out_imag


# revision 6
# speedup vs baseline: 1.0021x; 1.0008x over previous
"""Bass/Trainium2 kernel for nn_BasicQuantumAttention (B=4, L=2048, d=512, 8 cores).

Sharding: core (b, s) = batch b, stream s (real/imag). Per core, fully fused
single-pass bf16 pipeline (f32 PSUM accumulation everywhere):
  - projection computes q^T, k^T (via W^T as lhsT over x^T) and v directly
    into resident SBUF -- no transposes, no DRAM round-trip
  - block-sparse masked attention (compile-time tile skipping over the union
    of the two masks; all 8 cores share one program)
  - partial out-projection y^T_part = W_out^T[stream rows].T @ O_norm^T
Host sums the two partial y^T per batch and untransposes.

Projection slices (256 tokens) are interleaved with attention/out-projection
blocks in emission order so softmax reciprocal chains hide under projection
matmuls and the PE never idles between phases.
"""
import sys

sys.path.insert(0, "/opt/trn_rl_repo")

import numpy as np
from ml_dtypes import bfloat16

import concourse.bass as bass
import concourse.tile as tile
from concourse import bacc, bass_isa, mybir
from concourse.bass_utils import run_bass_kernel_spmd

B, L, D = 4, 2048, 512
C6 = 6 * D            # 3072 input features
CT = C6 // 128        # 24 contraction tiles
NT = L // 128         # 16 token tiles of 128
NS = L // 256         # 8 projection slices of 256 tokens
QS = L // 512         # 4 query slices of 512 (out-projection granularity)
QB = L // 128         # 16 query blocks of 128 (attention granularity)
KT = L // 128         # 16 key tiles of 128
F32 = mybir.dt.float32
BF = mybir.dt.bfloat16
SCALE = float(D) ** -0.5
IDENT = mybir.ActivationFunctionType.Identity
EXP = mybir.ActivationFunctionType.Exp

# feature offsets inside qkv = [q_r q_i k_r k_i v_r v_i] (each D wide)
_Q_OFF = {0: 0 * D, 1: 1 * D}
_K_OFF = {0: 2 * D, 1: 3 * D}
_V_OFF = {0: 4 * D, 1: 5 * D}

LAST_RESULTS = None  # for test harness introspection


def build_program(kept, needs_mask, slot_of, width, n_slots, n_reps=1):
    """kept: {qs: [kt,...]} union keep lists; needs_mask: set[(qs,kt)];
    slot_of: {(qs,kt): slot index}; n_slots >= 1."""
    nc = bacc.Bacc(None, target_bir_lowering=False, debug=False)

    x_t = nc.dram_tensor("x_t", [128, NS, CT, 256], BF, kind="ExternalInput")
    wq = nc.dram_tensor("wq", [128, CT, D], BF, kind="ExternalInput")
    wk = nc.dram_tensor("wk", [128, CT, D], BF, kind="ExternalInput")
    wv = nc.dram_tensor("wv", [128, CT, D], BF, kind="ExternalInput")
    wo = nc.dram_tensor("wo", [128, 4, 2 * D], BF, kind="ExternalInput")
    b_qk = nc.dram_tensor("b_qk", [128, 8], F32, kind="ExternalInput")
    b_y = nc.dram_tensor("b_y", [128, 8], F32, kind="ExternalInput")
    mask_t = nc.dram_tensor("mask_t", [n_slots, 128, 128], BF, kind="ExternalInput")
    y = nc.dram_tensor("y", [2 * D, L], F32, kind="ExternalOutput")

    with tile.TileContext(nc) as tc, \
         nc.allow_low_precision(reason="bf16 matmul inputs, f32 accumulation"), \
         tc.tile_pool(name="consts", bufs=1) as consts, \
         tc.tile_pool(name="xin", bufs=3) as xp, \
         tc.tile_pool(name="pp", bufs=2, space="PSUM") as pp, \
         tc.tile_pool(name="sy", bufs=2, space="PSUM") as syp, \
         tc.tile_pool(name="op", bufs=1, space="PSUM") as opp, \
         tc.tile_pool(name="pt", bufs=4) as ptp, \
         tc.tile_pool(name="mk", bufs=12) as mkp, \
         tc.tile_pool(name="ot", bufs=2) as otp, \
         tc.tile_pool(name="dc", bufs=3) as dcp, \
         tc.tile_pool(name="oc", bufs=2) as ocp, \
         tc.tile_pool(name="rc", bufs=2) as rcp, \
         tc.tile_pool(name="rb", bufs=2) as rbp, \
         tc.tile_pool(name="yo", bufs=3) as yop:
        wq_sb = consts.tile([128, CT, D], BF)
        wk_sb = consts.tile([128, CT, D], BF)
        wv_sb = consts.tile([128, CT, D], BF)
        wo_sb = consts.tile([128, 4, 2 * D], BF)
        bqk_sb = consts.tile([128, 8], F32)
        by_sb = consts.tile([128, 8], F32)

        def load_x(ns2, eng=None, splits=(12, 24)):
            xt = xp.tile([128, CT, 256], BF, tag="x", name="xt")
            h0 = 0
            for h1 in splits:
                (eng or nc.sync).dma_start(
                    out=xt[:, h0:h1, :],
                    in_=x_t[:, ns2, h0:h1, :])
                h0 = h1
            return xt

        # x0/x1 go out on the Act queue (idle at startup) so SP and Pool can
        # stream weight chunks from t=0; all run ahead of first use. x0's
        # first piece is small so the very first matmul unblocks early.
        x_tiles = {0: load_x(0, eng=nc.scalar, splits=(4, 12, 24)),
                   1: load_x(1, eng=nc.scalar)}

        # bias first (it gates the first PSUM evacuation), then weights in
        # chunks alternating Pool/SP so arrival order tracks the ct
        # consumption order; non-critical consts follow the weights
        nc.gpsimd.dma_start(out=bqk_sb, in_=b_qk[:, :])
        wq_chunks = [(0, 2), (2, 4), (4, 8), (8, 12), (12, 16), (16, 20),
                     (20, 24)]
        for i, (c0, c1) in enumerate(wq_chunks):
            eng = nc.gpsimd if i % 2 == 0 else nc.sync
            eng.dma_start(out=wq_sb[:, c0:c1, :], in_=wq[:, c0:c1, :])
        for w_dram, w_sb in ((wk, wk_sb), (wv, wv_sb)):
            for i, c0 in enumerate(range(0, CT, 4)):
                eng = nc.gpsimd if i % 2 == 0 else nc.sync
                eng.dma_start(out=w_sb[:, c0:c0 + 4, :],
                              in_=w_dram[:, c0:c0 + 4, :])
        nc.gpsimd.dma_start(out=wo_sb, in_=wo[:, :, :])
        nc.gpsimd.dma_start(out=by_sb, in_=b_y[:, :])
        # warmup: trigger the one-time activation function-table load off
        # the critical path, before the first PSUM evacuation needs it
        warm = consts.tile([1, 1], F32)
        nc.scalar.activation(out=warm, in_=bqk_sb[0:1, 0:1], func=EXP)

        qT_sb = consts.tile([128, 4, L], BF, name="qT")
        kT_sb = consts.tile([128, 4, L], BF, name="kT")
        v_sb = consts.tile([128, NT, D], BF, name="v")

        for _rep in range(n_reps):
            if _rep > 0:
                x_tiles = {0: load_x(0), 1: load_x(1)}

            def emit_P(ns2):
                xt = x_tiles.pop(ns2)
                if ns2 + 2 < NS:
                    x_tiles[ns2 + 2] = load_x(ns2 + 2)
                for which, w_sb, dst in (("q", wq_sb, qT_sb),
                                         ("k", wk_sb, kT_sb)):
                    for dt in range(4):
                        ps = pp.tile([128, D], F32, tag="pp")
                        for ct in range(CT):
                            nc.tensor.matmul(
                                ps[:, 0:256],
                                w_sb[:, ct, dt * 128:(dt + 1) * 128],
                                xt[:, ct, :],
                                start=(ct == 0), stop=(ct == CT - 1))
                        bc = dt if which == "q" else 4 + dt
                        nc.scalar.activation(
                            out=dst[:, dt, ns2 * 256:(ns2 + 1) * 256],
                            in_=ps[:, 0:256], func=IDENT,
                            bias=bqk_sb[:, bc:bc + 1])
                for nt2 in range(2):
                    nt = ns2 * 2 + nt2
                    ps = pp.tile([128, D], F32, tag="pp")
                    for ct in range(CT):
                        nc.tensor.matmul(
                            ps[:, :], xt[:, ct, nt2 * 128:(nt2 + 1) * 128],
                            wv_sb[:, ct, :],
                            start=(ct == 0), stop=(ct == CT - 1))
                    nc.scalar.copy(out=v_sb[:, nt, :], in_=ps)

            otqs = {}

            def emit_A(qs, filler=None):
                # 4 query blocks of 128, each with its own softmax chain,
                # normalized into one [128, 4dvt, 512] tile for emit_O.
                # filler(j) emits independent PE work between blocks so the
                # per-block softmax chains stay hidden even with no
                # projection work left (used for the final A group).
                otq = otp.tile([128, 4, 512], BF, tag="otq", name="otq")
                otqs[qs] = otq
                for qb in range(4 * qs, 4 * qs + 4):
                    if filler is not None and qb > 4 * qs:
                        filler(qb - 4 * qs - 1)
                    klist = kept[qb]
                    # PSUM is bank-granular (2 KB): pack the four [128,128]
                    # AV accumulators and the rotating score tiles into
                    # [128,512] bank tiles, addressed by 128-col slices
                    # one open accumulation group per PSUM bank: the four AV
                    # accumulators and each rotating score tile get their own
                    # bank-aligned tiles
                    ops = [opp.tile([128, 128], F32, tag=f"o{dvt}",
                                    name=f"ops{dvt}") for dvt in range(4)]
                    dacc = dcp.tile([128, 128], F32, name="dacc")
                    # software-pipelined: scores for tile i+1 are emitted
                    # before the AV matmuls of tile i, so the PE streams
                    # through the exp->mask latency of each tile
                    n_k = len(klist)
                    pend = []  # (i, kt, pT, c0, cw) awaiting AV matmuls

                    def emit_AV(i, kt, pT, c0, cw):
                        for dvt in range(4):
                            nc.tensor.matmul(
                                ops[dvt][:, c0:c0 + cw],
                                v_sb[:, kt, dvt * 128:(dvt + 1) * 128],
                                pT[:, c0:c0 + cw],
                                start=(i == 0), stop=(i == n_k - 1))

                    for i, kt in enumerate(klist):
                        c0, cw = width[(qb, kt)]
                        sps = syp.tile([128, 128], F32, tag="sy", name="sps")
                        for dt in range(4):
                            nc.tensor.matmul(
                                sps[:, c0:c0 + cw],
                                kT_sb[:, dt, kt * 128:(kt + 1) * 128],
                                qT_sb[:, dt, qb * 128 + c0:qb * 128 + c0 + cw],
                                start=(dt == 0), stop=(dt == 3))
                        pT = ptp.tile([128, 128], BF, name="pT")
                        nc.scalar.activation(out=pT[:, c0:c0 + cw],
                                             in_=sps[:, c0:c0 + cw], func=EXP,
                                             scale=SCALE)
                        if (qb, kt) in needs_mask:
                            mt = mkp.tile([128, 128], BF, name="mt")
                            nc.sync.dma_start(
                                out=mt[:, c0:c0 + cw],
                                in_=mask_t[slot_of[(qb, kt)], :, c0:c0 + cw])
                            nc.vector.tensor_mul(pT[:, c0:c0 + cw],
                                                 pT[:, c0:c0 + cw],
                                                 mt[:, c0:c0 + cw])
                        if i == 0:
                            # diagonal tile: always full width, initializes
                            # the whole accumulator
                            nc.vector.tensor_copy(out=dacc, in_=pT)
                        else:
                            nc.vector.tensor_add(dacc[:, c0:c0 + cw],
                                                 dacc[:, c0:c0 + cw],
                                                 pT[:, c0:c0 + cw])
                        pend.append((i, kt, pT, c0, cw))
                        if len(pend) > 2:
                            emit_AV(*pend.pop(0))
                    for p_ in pend:
                        emit_AV(*p_)
                    # evacuate the AV accumulators with plain copies first:
                    # this releases the PSUM banks for the next block without
                    # waiting on the reciprocal chain
                    oc = [ocp.tile([128, 128], BF, tag=f"c{dvt}",
                                   name=f"oc{dvt}") for dvt in range(4)]
                    for dvt in range(4):
                        nc.vector.tensor_copy(out=oc[dvt], in_=ops[dvt][:, :])
                    # denominator: all-reduce across partitions on gpsimd
                    # (reduce + broadcast in one op, PE stays out of it),
                    # then reciprocal on DVE and deferred normalization
                    den = rcp.tile([128, 128], F32, name="den")
                    nc.gpsimd.partition_all_reduce(
                        den, dacc, channels=128,
                        reduce_op=bass_isa.ReduceOp.add)
                    rb = rbp.tile([128, 128], F32, name="rb")
                    nc.vector.reciprocal(rb, den)
                    qo = (qb - 4 * qs) * 128
                    for dvt in range(4):
                        nc.vector.tensor_mul(
                            otq[:, dvt, qo:qo + 128], oc[dvt], rb)

            def emit_O(qs, split=False):
                # split=True accumulates each yps tile in four sequential
                # 128-col piece-groups (one open group per bank at a time),
                # so the first pieces only depend on the first attention
                # blocks' normalization — the last O block starts while the
                # final softmax chains are still draining
                for gt in range(8):
                    yps = pp.tile([128, D], F32, tag="pp", name="yps")
                    if split:
                        for qp in range(4):
                            for dvt in range(4):
                                nc.tensor.matmul(
                                    yps[:, qp * 128:(qp + 1) * 128],
                                    wo_sb[:, dvt, gt * 128:(gt + 1) * 128],
                                    otqs[qs][:, dvt, qp * 128:(qp + 1) * 128],
                                    start=(dvt == 0), stop=(dvt == 3))
                    else:
                        for dvt in range(4):
                            nc.tensor.matmul(
                                yps[:, :], wo_sb[:, dvt, gt * 128:(gt + 1) * 128],
                                otqs[qs][:, dvt, :],
                                start=(dvt == 0), stop=(dvt == 3))
                    y_sb = yop.tile([128, 512], F32, name="ysb")
                    nc.scalar.activation(out=y_sb, in_=yps, func=IDENT,
                                         bias=by_sb[:, gt:gt + 1])
                    nc.sync.dma_start(
                        out=y[gt * 128:(gt + 1) * 128,
                              qs * 512:(qs + 1) * 512],
                        in_=y_sb)

            emit_P(0); emit_P(1)
            emit_A(0)
            emit_P(2); emit_P(3)
            emit_A(1); emit_O(0)
            emit_P(4); emit_P(5)
            emit_A(2); emit_O(1)
            emit_P(6); emit_P(7)
            emit_A(3); emit_O(2, split=True); emit_O(3, split=True)

    nc.compile()
    return nc


def _prep_masks(mask_real, mask_imag):
    """Compile-time tile analysis at [128 keys, 128 queries] granularity
    over the union of the two stream masks + per-core mask slot data.

    Each kept (qb, kt) tile carries a query-column range (c0, cw): full
    width (0, 128) or one 64-wide half when the union mask is empty on the
    other half. The always-full-width diagonal tile is moved to the front
    of each klist so the first AV matmul of a block opens the PSUM
    accumulation group over the full bank."""
    mts = [np.ascontiguousarray(np.asarray(m).T) for m in (mask_real, mask_imag)]
    kept = {}
    needs_mask = set()
    slot_of = {}
    width = {}
    slots = []  # (qb, kt)
    for qb in range(QB):
        klist = []
        for kt in range(KT):
            subs = [m[kt * 128:(kt + 1) * 128, qb * 128:(qb + 1) * 128] for m in mts]
            lo = any(s[:, :64].any() for s in subs)
            hi = any(s[:, 64:].any() for s in subs)
            if not (lo or hi):
                continue
            klist.append(kt)
            if kt == qb or (lo and hi):
                c0, cw = 0, 128
            elif lo:
                c0, cw = 0, 64
            else:
                c0, cw = 64, 64
            width[(qb, kt)] = (c0, cw)
            if not all(s[:, c0:c0 + cw].all() for s in subs):
                needs_mask.add((qb, kt))
                slot_of[(qb, kt)] = len(slots)
                slots.append((qb, kt))
        # diagonal tile first: it is always kept and always full width
        klist.remove(qb)
        klist.insert(0, qb)
        kept[qb] = klist
    n_slots = max(1, len(slots))
    mask_data = []
    for s in range(2):
        md = np.ones((n_slots, 128, 128), bfloat16)
        for i, (qb, kt) in enumerate(slots):
            md[i] = mts[s][kt * 128:(kt + 1) * 128,
                           qb * 128:(qb + 1) * 128].astype(bfloat16)
        mask_data.append(md)
    return kept, needs_mask, slot_of, width, n_slots, mask_data


def kernel(q_real, q_imag, k_real, k_imag, v_real, v_imag,
           W_qkv, b_qkv, W_out, b_out, mask_real, mask_imag, _trace=False):
    global LAST_RESULTS
    args = [np.asarray(a) for a in (q_real, q_imag, k_real, k_imag, v_real, v_imag)]
    W_qkv = np.asarray(W_qkv, np.float32)
    b_qkv = np.asarray(b_qkv, np.float32)
    W_out = np.asarray(W_out, np.float32)
    b_out = np.asarray(b_out, np.float32)

    kept, needs_mask, slot_of, width, n_slots, mask_data = _prep_masks(
        mask_real, mask_imag)
    nc = build_program(kept, needs_mask, slot_of, width, n_slots)

    # x^T per batch, partition-major: [128, CT, L]
    x_ts = []
    for b in range(B):
        xb = np.concatenate([a[b] for a in args], axis=1)           # [L, 6D]
        xt = xb.T.astype(bfloat16)                                  # [6D, L]
        x_ts.append(np.ascontiguousarray(
            xt.reshape(CT, 128, NS, 256).transpose(1, 2, 0, 3)))    # [128,NS,CT,256]

    W6T = W_qkv.T  # [c, f]
    W2T = W_out.T  # [f=2D, g=2D]
    wqs, wks, wvs, wos, b_qks, b_ys = [], [], [], [], [], []
    for s in range(2):
        wqs.append(np.ascontiguousarray(
            W6T[:, _Q_OFF[s]:_Q_OFF[s] + D].astype(bfloat16)
            .reshape(CT, 128, D).transpose(1, 0, 2)))               # [128,CT,D]
        wks.append(np.ascontiguousarray(
            W6T[:, _K_OFF[s]:_K_OFF[s] + D].astype(bfloat16)
            .reshape(CT, 128, D).transpose(1, 0, 2)))
        wvs.append(np.ascontiguousarray(
            W6T[:, _V_OFF[s]:_V_OFF[s] + D].astype(bfloat16)
            .reshape(CT, 128, D).transpose(1, 0, 2)))
        wos.append(np.ascontiguousarray(
            W2T[s * D:(s + 1) * D, :].astype(bfloat16)
            .reshape(4, 128, 2 * D).transpose(1, 0, 2)))            # [128,4,1024]
        bq = b_qkv[_Q_OFF[s]:_Q_OFF[s] + D].reshape(4, 128).T
        bk = b_qkv[_K_OFF[s]:_K_OFF[s] + D].reshape(4, 128).T
        b_qks.append(np.ascontiguousarray(
            np.concatenate([bq, bk], axis=1), dtype=np.float32))    # [128, 8]
        if s == 0:
            b_v_cat = np.concatenate([b_qkv[_V_OFF[0]:_V_OFF[0] + D],
                                      b_qkv[_V_OFF[1]:_V_OFF[1] + D]])
            b_eff = (W_out @ b_v_cat + b_out).astype(np.float32)
            b_ys.append(np.ascontiguousarray(b_eff.reshape(8, 128).T))
        else:
            b_ys.append(np.zeros((128, 8), np.float32))

    in_maps = []
    for core in range(8):
        b, s = core // 2, core % 2
        in_maps.append({
            "x_t": x_ts[b], "wq": wqs[s], "wk": wks[s], "wv": wvs[s],
            "wo": wos[s], "b_qk": b_qks[s], "b_y": b_ys[s],
            "mask_t": mask_data[s],
        })

    res = run_bass_kernel_spmd(nc, in_maps, core_ids=list(range(8)), trace=_trace)
    LAST_RESULTS = res

    out_real = np.empty((B, L, D), np.float32)
    out_imag = np.empty((B, L, D), np.float32)
    for b in range(B):
        yt = res.results[2 * b]["y"] + res.results[2 * b + 1]["y"]  # [2D, L]
        yb = yt.T                                                   # [L, 2D]
        out_real[b] = yb[:, :D]
        out_imag[b] = yb[:, D:]
    return out_real, out_imag


# revision 7
# speedup vs baseline: 1.0035x; 1.0014x over previous
"""Bass/Trainium2 kernel for nn_BasicQuantumAttention (B=4, L=2048, d=512, 8 cores).

Sharding: core (b, s) = batch b, stream s (real/imag). Per core, fully fused
single-pass bf16 pipeline (f32 PSUM accumulation everywhere):
  - projection computes q^T, k^T (via W^T as lhsT over x^T) and v directly
    into resident SBUF -- no transposes, no DRAM round-trip
  - block-sparse masked attention (compile-time tile skipping over the union
    of the two masks; all 8 cores share one program)
  - partial out-projection y^T_part = W_out^T[stream rows].T @ O_norm^T
Host sums the two partial y^T per batch and untransposes.

Projection slices (256 tokens) are interleaved with attention/out-projection
blocks in emission order so softmax reciprocal chains hide under projection
matmuls and the PE never idles between phases.
"""
import sys

sys.path.insert(0, "/opt/trn_rl_repo")

import numpy as np
from ml_dtypes import bfloat16

import concourse.bass as bass
import concourse.tile as tile
from concourse import bacc, bass_isa, mybir
from concourse.bass_utils import run_bass_kernel_spmd

B, L, D = 4, 2048, 512
C6 = 6 * D            # 3072 input features
CT = C6 // 128        # 24 contraction tiles
NT = L // 128         # 16 token tiles of 128
NS = L // 256         # 8 projection slices of 256 tokens
QS = L // 512         # 4 query slices of 512 (out-projection granularity)
QB = L // 128         # 16 query blocks of 128 (attention granularity)
KT = L // 128         # 16 key tiles of 128
F32 = mybir.dt.float32
BF = mybir.dt.bfloat16
SCALE = float(D) ** -0.5
IDENT = mybir.ActivationFunctionType.Identity
EXP = mybir.ActivationFunctionType.Exp

# feature offsets inside qkv = [q_r q_i k_r k_i v_r v_i] (each D wide)
_Q_OFF = {0: 0 * D, 1: 1 * D}
_K_OFF = {0: 2 * D, 1: 3 * D}
_V_OFF = {0: 4 * D, 1: 5 * D}

LAST_RESULTS = None  # for test harness introspection


def build_program(kept, needs_mask, slot_of, width, n_slots, n_reps=1):
    """kept: {qs: [kt,...]} union keep lists; needs_mask: set[(qs,kt)];
    slot_of: {(qs,kt): slot index}; n_slots >= 1."""
    nc = bacc.Bacc(None, target_bir_lowering=False, debug=False)

    x_t = nc.dram_tensor("x_t", [128, NS, CT, 256], BF, kind="ExternalInput")
    wq = nc.dram_tensor("wq", [128, CT, D], BF, kind="ExternalInput")
    wk = nc.dram_tensor("wk", [128, CT, D], BF, kind="ExternalInput")
    wv = nc.dram_tensor("wv", [128, CT, D], BF, kind="ExternalInput")
    wo = nc.dram_tensor("wo", [128, 4, 2 * D], BF, kind="ExternalInput")
    b_qk = nc.dram_tensor("b_qk", [128, 8], F32, kind="ExternalInput")
    b_y = nc.dram_tensor("b_y", [128, 8], F32, kind="ExternalInput")
    mask_t = nc.dram_tensor("mask_t", [n_slots, 128, 128], BF, kind="ExternalInput")
    y = nc.dram_tensor("y", [2 * D, L], F32, kind="ExternalOutput")

    with tile.TileContext(nc) as tc, \
         nc.allow_low_precision(reason="bf16 matmul inputs, f32 accumulation"), \
         tc.tile_pool(name="consts", bufs=1) as consts, \
         tc.tile_pool(name="xin", bufs=4) as xp, \
         tc.tile_pool(name="pp", bufs=2, space="PSUM") as pp, \
         tc.tile_pool(name="sy", bufs=2, space="PSUM") as syp, \
         tc.tile_pool(name="op", bufs=1, space="PSUM") as opp, \
         tc.tile_pool(name="pt", bufs=4) as ptp, \
         tc.tile_pool(name="mk", bufs=12) as mkp, \
         tc.tile_pool(name="ot", bufs=2) as otp, \
         tc.tile_pool(name="dc", bufs=3) as dcp, \
         tc.tile_pool(name="oc", bufs=2) as ocp, \
         tc.tile_pool(name="rc", bufs=2) as rcp, \
         tc.tile_pool(name="rb", bufs=2) as rbp, \
         tc.tile_pool(name="yo", bufs=4) as yop:
        wq_sb = consts.tile([128, CT, D], BF)
        wk_sb = consts.tile([128, CT, D], BF)
        wv_sb = consts.tile([128, CT, D], BF)
        wo_sb = consts.tile([128, 4, 2 * D], BF)
        bqk_sb = consts.tile([128, 8], F32)
        by_sb = consts.tile([128, 8], F32)

        def load_x(ns2, eng=None, splits=(12, 24)):
            xt = xp.tile([128, CT, 256], BF, tag="x", name="xt")
            h0 = 0
            for h1 in splits:
                (eng or nc.sync).dma_start(
                    out=xt[:, h0:h1, :],
                    in_=x_t[:, ns2, h0:h1, :])
                h0 = h1
            return xt

        # x0/x1 go out on the Act queue (idle at startup) so SP and Pool can
        # stream weight chunks from t=0; all run ahead of first use. x0's
        # first piece is small so the very first matmul unblocks early.
        x_tiles = {0: load_x(0, eng=nc.scalar, splits=(4, 12, 24)),
                   1: load_x(1, eng=nc.scalar)}

        # bias first (it gates the first PSUM evacuation), then weights in
        # chunks alternating Pool/SP so arrival order tracks the ct
        # consumption order; non-critical consts follow the weights
        nc.gpsimd.dma_start(out=bqk_sb, in_=b_qk[:, :])
        wq_chunks = [(0, 2), (2, 4), (4, 8), (8, 12), (12, 16), (16, 20),
                     (20, 24)]
        for i, (c0, c1) in enumerate(wq_chunks):
            eng = nc.gpsimd if i % 2 == 0 else nc.sync
            eng.dma_start(out=wq_sb[:, c0:c1, :], in_=wq[:, c0:c1, :])
        for w_dram, w_sb in ((wk, wk_sb), (wv, wv_sb)):
            for i, c0 in enumerate(range(0, CT, 4)):
                eng = nc.gpsimd if i % 2 == 0 else nc.sync
                eng.dma_start(out=w_sb[:, c0:c0 + 4, :],
                              in_=w_dram[:, c0:c0 + 4, :])
        nc.gpsimd.dma_start(out=wo_sb, in_=wo[:, :, :])
        nc.gpsimd.dma_start(out=by_sb, in_=b_y[:, :])
        # warmup: trigger the one-time activation function-table load off
        # the critical path, before the first PSUM evacuation needs it
        warm = consts.tile([1, 1], F32)
        nc.scalar.activation(out=warm, in_=bqk_sb[0:1, 0:1], func=EXP)

        qT_sb = consts.tile([128, 4, L], BF, name="qT")
        kT_sb = consts.tile([128, 4, L], BF, name="kT")
        v_sb = consts.tile([128, NT, D], BF, name="v")

        for _rep in range(n_reps):
            if _rep > 0:
                x_tiles = {0: load_x(0), 1: load_x(1)}

            def emit_P(ns2):
                xt = x_tiles.pop(ns2)
                if ns2 + 2 < NS:
                    x_tiles[ns2 + 2] = load_x(ns2 + 2)
                for which, w_sb, dst in (("q", wq_sb, qT_sb),
                                         ("k", wk_sb, kT_sb)):
                    for dt in range(4):
                        ps = pp.tile([128, D], F32, tag="pp")
                        for ct in range(CT):
                            nc.tensor.matmul(
                                ps[:, 0:256],
                                w_sb[:, ct, dt * 128:(dt + 1) * 128],
                                xt[:, ct, :],
                                start=(ct == 0), stop=(ct == CT - 1))
                        bc = dt if which == "q" else 4 + dt
                        nc.scalar.activation(
                            out=dst[:, dt, ns2 * 256:(ns2 + 1) * 256],
                            in_=ps[:, 0:256], func=IDENT,
                            bias=bqk_sb[:, bc:bc + 1])
                for nt2 in range(2):
                    nt = ns2 * 2 + nt2
                    ps = pp.tile([128, D], F32, tag="pp")
                    for ct in range(CT):
                        nc.tensor.matmul(
                            ps[:, :], xt[:, ct, nt2 * 128:(nt2 + 1) * 128],
                            wv_sb[:, ct, :],
                            start=(ct == 0), stop=(ct == CT - 1))
                    nc.scalar.copy(out=v_sb[:, nt, :], in_=ps)

            otqs = {}

            def emit_A(qs, filler=None):
                # 4 query blocks of 128, each with its own softmax chain,
                # normalized into one [128, 4dvt, 512] tile for emit_O.
                # filler(j) emits independent PE work between blocks so the
                # per-block softmax chains stay hidden even with no
                # projection work left (used for the final A group).
                otq = otp.tile([128, 4, 512], BF, tag="otq", name="otq")
                otqs[qs] = otq
                for qb in range(4 * qs, 4 * qs + 4):
                    if filler is not None and qb > 4 * qs:
                        filler(qb - 4 * qs - 1)
                    klist = kept[qb]
                    # PSUM is bank-granular (2 KB): pack the four [128,128]
                    # AV accumulators and the rotating score tiles into
                    # [128,512] bank tiles, addressed by 128-col slices
                    # one open accumulation group per PSUM bank: the four AV
                    # accumulators and each rotating score tile get their own
                    # bank-aligned tiles
                    ops = [opp.tile([128, 128], F32, tag=f"o{dvt}",
                                    name=f"ops{dvt}") for dvt in range(4)]
                    dacc = dcp.tile([128, 128], F32, name="dacc")
                    # software-pipelined: scores for tile i+1 are emitted
                    # before the AV matmuls of tile i, so the PE streams
                    # through the exp->mask latency of each tile
                    n_k = len(klist)
                    pend = []  # (i, kt, pT, c0, cw) awaiting AV matmuls

                    def emit_AV(i, kt, pT, c0, cw):
                        for dvt in range(4):
                            nc.tensor.matmul(
                                ops[dvt][:, c0:c0 + cw],
                                v_sb[:, kt, dvt * 128:(dvt + 1) * 128],
                                pT[:, c0:c0 + cw],
                                start=(i == 0), stop=(i == n_k - 1))

                    for i, kt in enumerate(klist):
                        c0, cw = width[(qb, kt)]
                        sps = syp.tile([128, 128], F32, tag="sy", name="sps")
                        for dt in range(4):
                            nc.tensor.matmul(
                                sps[:, c0:c0 + cw],
                                kT_sb[:, dt, kt * 128:(kt + 1) * 128],
                                qT_sb[:, dt, qb * 128 + c0:qb * 128 + c0 + cw],
                                start=(dt == 0), stop=(dt == 3))
                        pT = ptp.tile([128, 128], BF, name="pT")
                        nc.scalar.activation(out=pT[:, c0:c0 + cw],
                                             in_=sps[:, c0:c0 + cw], func=EXP,
                                             scale=SCALE)
                        if (qb, kt) in needs_mask:
                            mt = mkp.tile([128, 128], BF, name="mt")
                            nc.sync.dma_start(
                                out=mt[:, c0:c0 + cw],
                                in_=mask_t[slot_of[(qb, kt)], :, c0:c0 + cw])
                            nc.vector.tensor_mul(pT[:, c0:c0 + cw],
                                                 pT[:, c0:c0 + cw],
                                                 mt[:, c0:c0 + cw])
                        if i == 0:
                            # diagonal tile: always full width, initializes
                            # the whole accumulator
                            nc.vector.tensor_copy(out=dacc, in_=pT)
                        else:
                            nc.vector.tensor_add(dacc[:, c0:c0 + cw],
                                                 dacc[:, c0:c0 + cw],
                                                 pT[:, c0:c0 + cw])
                        pend.append((i, kt, pT, c0, cw))
                        if len(pend) > 2:
                            emit_AV(*pend.pop(0))
                    for p_ in pend:
                        emit_AV(*p_)
                    # evacuate the AV accumulators with plain copies first:
                    # this releases the PSUM banks for the next block without
                    # waiting on the reciprocal chain
                    oc = [ocp.tile([128, 128], BF, tag=f"c{dvt}",
                                   name=f"oc{dvt}") for dvt in range(4)]
                    for dvt in range(4):
                        nc.vector.tensor_copy(out=oc[dvt], in_=ops[dvt][:, :])
                    # denominator: all-reduce across partitions on gpsimd
                    # (reduce + broadcast in one op, PE stays out of it),
                    # then reciprocal on DVE and deferred normalization
                    den = rcp.tile([128, 128], F32, name="den")
                    nc.gpsimd.partition_all_reduce(
                        den, dacc, channels=128,
                        reduce_op=bass_isa.ReduceOp.add)
                    rb = rbp.tile([128, 128], F32, name="rb")
                    nc.vector.reciprocal(rb, den)
                    qo = (qb - 4 * qs) * 128
                    for dvt in range(4):
                        nc.vector.tensor_mul(
                            otq[:, dvt, qo:qo + 128], oc[dvt], rb)

            def emit_O(qs, split=False):
                # split=True accumulates each yps tile in four sequential
                # 128-col piece-groups (one open group per bank at a time),
                # so the first pieces only depend on the first attention
                # blocks' normalization — the last O block starts while the
                # final softmax chains are still draining
                for gt in range(8):
                    yps = pp.tile([128, D], F32, tag="pp", name="yps")
                    if split:
                        for qp in range(4):
                            for dvt in range(4):
                                nc.tensor.matmul(
                                    yps[:, qp * 128:(qp + 1) * 128],
                                    wo_sb[:, dvt, gt * 128:(gt + 1) * 128],
                                    otqs[qs][:, dvt, qp * 128:(qp + 1) * 128],
                                    start=(dvt == 0), stop=(dvt == 3))
                    else:
                        for dvt in range(4):
                            nc.tensor.matmul(
                                yps[:, :], wo_sb[:, dvt, gt * 128:(gt + 1) * 128],
                                otqs[qs][:, dvt, :],
                                start=(dvt == 0), stop=(dvt == 3))
                    y_sb = yop.tile([128, 512], F32, name="ysb")
                    nc.scalar.activation(out=y_sb, in_=yps, func=IDENT,
                                         bias=by_sb[:, gt:gt + 1])
                    nc.sync.dma_start(
                        out=y[gt * 128:(gt + 1) * 128,
                              qs * 512:(qs + 1) * 512],
                        in_=y_sb)

            emit_P(0); emit_P(1)
            emit_A(0)
            emit_P(2); emit_P(3)
            emit_A(1); emit_O(0)
            emit_P(4); emit_P(5)
            emit_A(2); emit_O(1)
            emit_P(6); emit_P(7)
            emit_A(3); emit_O(2, split=True); emit_O(3, split=True)

    nc.compile()
    return nc


def _prep_masks(mask_real, mask_imag):
    """Compile-time tile analysis at [128 keys, 128 queries] granularity
    over the union of the two stream masks + per-core mask slot data.

    Each kept (qb, kt) tile carries a query-column range (c0, cw): full
    width (0, 128) or one 64-wide half when the union mask is empty on the
    other half. The always-full-width diagonal tile is moved to the front
    of each klist so the first AV matmul of a block opens the PSUM
    accumulation group over the full bank."""
    mts = [np.ascontiguousarray(np.asarray(m).T) for m in (mask_real, mask_imag)]
    kept = {}
    needs_mask = set()
    slot_of = {}
    width = {}
    slots = []  # (qb, kt)
    for qb in range(QB):
        klist = []
        for kt in range(KT):
            subs = [m[kt * 128:(kt + 1) * 128, qb * 128:(qb + 1) * 128] for m in mts]
            lo = any(s[:, :64].any() for s in subs)
            hi = any(s[:, 64:].any() for s in subs)
            if not (lo or hi):
                continue
            klist.append(kt)
            if kt == qb or (lo and hi):
                c0, cw = 0, 128
            elif lo:
                c0, cw = 0, 64
            else:
                c0, cw = 64, 64
            width[(qb, kt)] = (c0, cw)
            if not all(s[:, c0:c0 + cw].all() for s in subs):
                needs_mask.add((qb, kt))
                slot_of[(qb, kt)] = len(slots)
                slots.append((qb, kt))
        # diagonal tile first: it is always kept and always full width
        klist.remove(qb)
        klist.insert(0, qb)
        kept[qb] = klist
    n_slots = max(1, len(slots))
    mask_data = []
    for s in range(2):
        md = np.ones((n_slots, 128, 128), bfloat16)
        for i, (qb, kt) in enumerate(slots):
            md[i] = mts[s][kt * 128:(kt + 1) * 128,
                           qb * 128:(qb + 1) * 128].astype(bfloat16)
        mask_data.append(md)
    return kept, needs_mask, slot_of, width, n_slots, mask_data


def kernel(q_real, q_imag, k_real, k_imag, v_real, v_imag,
           W_qkv, b_qkv, W_out, b_out, mask_real, mask_imag, _trace=False):
    global LAST_RESULTS
    args = [np.asarray(a) for a in (q_real, q_imag, k_real, k_imag, v_real, v_imag)]
    W_qkv = np.asarray(W_qkv, np.float32)
    b_qkv = np.asarray(b_qkv, np.float32)
    W_out = np.asarray(W_out, np.float32)
    b_out = np.asarray(b_out, np.float32)

    kept, needs_mask, slot_of, width, n_slots, mask_data = _prep_masks(
        mask_real, mask_imag)
    nc = build_program(kept, needs_mask, slot_of, width, n_slots)

    # x^T per batch, partition-major: [128, CT, L]
    x_ts = []
    for b in range(B):
        xb = np.concatenate([a[b] for a in args], axis=1)           # [L, 6D]
        xt = xb.T.astype(bfloat16)                                  # [6D, L]
        x_ts.append(np.ascontiguousarray(
            xt.reshape(CT, 128, NS, 256).transpose(1, 2, 0, 3)))    # [128,NS,CT,256]

    W6T = W_qkv.T  # [c, f]
    W2T = W_out.T  # [f=2D, g=2D]
    wqs, wks, wvs, wos, b_qks, b_ys = [], [], [], [], [], []
    for s in range(2):
        wqs.append(np.ascontiguousarray(
            W6T[:, _Q_OFF[s]:_Q_OFF[s] + D].astype(bfloat16)
            .reshape(CT, 128, D).transpose(1, 0, 2)))               # [128,CT,D]
        wks.append(np.ascontiguousarray(
            W6T[:, _K_OFF[s]:_K_OFF[s] + D].astype(bfloat16)
            .reshape(CT, 128, D).transpose(1, 0, 2)))
        wvs.append(np.ascontiguousarray(
            W6T[:, _V_OFF[s]:_V_OFF[s] + D].astype(bfloat16)
            .reshape(CT, 128, D).transpose(1, 0, 2)))
        wos.append(np.ascontiguousarray(
            W2T[s * D:(s + 1) * D, :].astype(bfloat16)
            .reshape(4, 128, 2 * D).transpose(1, 0, 2)))            # [128,4,1024]
        bq = b_qkv[_Q_OFF[s]:_Q_OFF[s] + D].reshape(4, 128).T
        bk = b_qkv[_K_OFF[s]:_K_OFF[s] + D].reshape(4, 128).T
        b_qks.append(np.ascontiguousarray(
            np.concatenate([bq, bk], axis=1), dtype=np.float32))    # [128, 8]
        if s == 0:
            b_v_cat = np.concatenate([b_qkv[_V_OFF[0]:_V_OFF[0] + D],
                                      b_qkv[_V_OFF[1]:_V_OFF[1] + D]])
            b_eff = (W_out @ b_v_cat + b_out).astype(np.float32)
            b_ys.append(np.ascontiguousarray(b_eff.reshape(8, 128).T))
        else:
            b_ys.append(np.zeros((128, 8), np.float32))

    in_maps = []
    for core in range(8):
        b, s = core // 2, core % 2
        in_maps.append({
            "x_t": x_ts[b], "wq": wqs[s], "wk": wks[s], "wv": wvs[s],
            "wo": wos[s], "b_qk": b_qks[s], "b_y": b_ys[s],
            "mask_t": mask_data[s],
        })

    res = run_bass_kernel_spmd(nc, in_maps, core_ids=list(range(8)), trace=_trace)
    LAST_RESULTS = res

    out_real = np.empty((B, L, D), np.float32)
    out_imag = np.empty((B, L, D), np.float32)
    for b in range(B):
        yt = res.results[2 * b]["y"] + res.results[2 * b + 1]["y"]  # [2D, L]
        yb = yt.T                                                   # [L, 2D]
        out_real[b] = yb[:, :D]
        out_imag[b] = yb[:, D:]
    return out_real, out_imag
